# revision 14
# baseline (speedup 1.0000x reference)
"""AKT (sparse attention) Trainium2 kernel — 8 NeuronCores.

Sharding: data-parallel over batch B=4 (pairs of cores share a batch) x
tensor-parallel over heads (each core owns 4 of the 8 heads). Attention
output projections produce per-core partial sums that are AllReduced
within each pair of cores; the final prediction row is AllReduced the
same way (tiny, [1,S]).

Math notes:
  - reference does softmax -> tril mask -> renormalize, which is exactly a
    masked (causal) softmax, so we only compute the lower-triangular blocks.
  - scores are tiny (|s| << 1), so no max-subtraction is needed for exp.
  - kr (strictly causal) row 0 has empty support; den + eps keeps it a clean
    zero, and the downstream shift discards that row anyway.

All matmuls run in bf16 with f32 PSUM accumulation (rel tolerance 2e-2;
values here are tiny and well-conditioned, measured rel err ~1e-4).
"""

import sys

if "/opt/trn_rl_repo" not in sys.path:
    sys.path.insert(0, "/opt/trn_rl_repo")

import numpy as np

import concourse.bass as bass
import concourse.bacc as bacc
import concourse.tile as tile
import concourse.mybir as mybir
from concourse.bass_utils import run_bass_kernel_spmd

dt = mybir.dt
AF = mybir.ActivationFunctionType
ALU = mybir.AluOpType

B, S, D, H = 4, 1024, 256, 8
P_TAB, C = 10000, 256
HL = H // 2          # heads per core
NT = S // 128        # 8 sequence tiles of 128
SC = 512             # free-dim chunk (one PSUM bank of f32)
NCH = S // SC        # 2 chunks


def _mha(nc, tc, pools, consts, qk_src, v_src, wq, wk, wv, wo, strict, ret_out):
    """One multi-head attention on 4 local heads.

    qk_src / v_src: SBUF bf16 [128, 2*S]  (transposed activations, free = kt*S + s)
    wq/wk/wv: SBUF bf16 [128, HL*2*256]   (free = h*512 + kt*256 + e)
    wo:       SBUF bf16 [128, 8*256]      (free = g*256 + d, g = local hDe tile)
    ret_out:  SBUF bf16 [128, 2*S]        (normalized+projected partial out, transposed)
    """
    work, ptp, psA, psAcc, psRow = (
        pools["work"], pools["pt"], pools["psA"], pools["psAcc"], pools["psRow"])
    ones_col, ones_row, mask = consts["ones_col"], consts["ones_row"], (
        consts["mask_s"] if strict else consts["mask_i"])

    rc = {}
    for h in range(HL):
        # --- projections ---
        qT = work.tile([128, 2 * S], dt.bfloat16, tag="qT")
        kT = work.tile([128, 2 * S], dt.bfloat16, tag="kT")
        for dst, w_sb in ((qT, wq), (kT, wk)):
            for mt in range(2):           # De half (output partition tile)
                for ch in range(NCH):
                    ps = psA.tile([128, SC], dt.float32, tag="mm")
                    for kt in range(2):   # D (contraction) half
                        nc.tensor.matmul(
                            ps[:],
                            w_sb[:, h * 512 + kt * 256 + mt * 128:
                                 h * 512 + kt * 256 + mt * 128 + 128],
                            qk_src[:, kt * S + ch * SC: kt * S + ch * SC + SC],
                            start=(kt == 0), stop=(kt == 1))
                    nc.any.tensor_copy(
                        dst[:, mt * S + ch * SC: mt * S + ch * SC + SC], ps[:])
        vN = work.tile([128, NT * 256], dt.bfloat16, tag="vN")
        for st in range(NT):              # V natural: [s-tile partitions, De free]
            ps = psA.tile([128, SC], dt.float32, tag="mm")
            for kt in range(2):
                nc.tensor.matmul(
                    ps[:, :256],
                    v_src[:, kt * S + st * 128: kt * S + st * 128 + 128],
                    wv[:, h * 512 + kt * 256: h * 512 + kt * 256 + 256],
                    start=(kt == 0), stop=(kt == 1))
            nc.any.tensor_copy(vN[:, st * 256: st * 256 + 256], ps[:, :256])

        # --- causal attention, transposed-softmax ---
        for mt in range(2):
            rc[(h, mt)] = work.tile([128, S], dt.bfloat16, tag=f"rc{h}{mt}",
                                    name=f"rc{h}{mt}", bufs=1)
        for qc in range(NCH):
            racc = [psAcc.tile([128, SC], dt.float32, tag="racc", name=f"racc{i}") for i in range(2)]
            dacc = psRow.tile([1, SC], dt.float32, tag="row")
            nk = 4 * (qc + 1)
            for kb in range(nk):
                n_off = max(0, kb * 128 - qc * SC)
                nq = SC - n_off
                q0 = qc * SC + n_off
                ps_s = psA.tile([128, SC], dt.float32, tag="mm")
                for kt in range(2):
                    nc.tensor.matmul(
                        ps_s[:, :nq],
                        kT[:, kt * S + kb * 128: kt * S + kb * 128 + 128],
                        qT[:, kt * S + q0: kt * S + q0 + nq],
                        start=(kt == 0), stop=(kt == 1))
                pt = ptp.tile([128, SC], dt.bfloat16, tag="pt")
                # exp(s/sqrt(D)); bf16 out for the AV matmul
                nc.scalar.activation(pt[:, :nq], ps_s[:, :nq], AF.Exp,
                                     scale=1.0 / 16.0)
                if kb >= qc * 4:  # diagonal 128-block is the first 128 cols
                    nc.vector.tensor_tensor(
                        out=pt[:, :128], in0=pt[:, :128], in1=mask[:],
                        op=ALU.mult)
                for mt in range(2):
                    nc.tensor.matmul(
                        racc[mt][:, n_off:SC],
                        vN[:, kb * 256 + mt * 128: kb * 256 + mt * 128 + 128],
                        pt[:, :nq],
                        start=(kb == 0), stop=(kb == nk - 1))
                nc.tensor.matmul(
                    dacc[:, n_off:SC], ones_col[:], pt[:, :nq],
                    start=(kb == 0), stop=(kb == nk - 1))
            # normalize: recip of row-sums, broadcast across partitions via K=1 matmul
            recip = ptp.tile([1, SC], dt.bfloat16, tag="recip", bufs=2)
            rec_f = ptp.tile([1, SC], dt.float32, tag="rec_f", bufs=2)
            if strict:
                den = ptp.tile([1, SC], dt.float32, tag="den", bufs=2)
                nc.vector.tensor_scalar_add(den[:], dacc[:], 1e-20)
                nc.vector.reciprocal(rec_f[:], den[:])
            else:
                nc.vector.reciprocal(rec_f[:], dacc[:])
            nc.vector.tensor_copy(recip[:], rec_f[:])
            bc = psA.tile([128, SC], dt.float32, tag="mm")
            nc.tensor.matmul(bc[:], ones_row[:1, :128], recip[:],
                             start=True, stop=True)
            bc_sb = ptp.tile([128, SC], dt.bfloat16, tag="bcs", bufs=2)
            nc.any.tensor_copy(bc_sb[:], bc[:])
            for mt in range(2):
                nc.vector.tensor_tensor(
                    out=rc[(h, mt)][:, qc * SC: qc * SC + SC],
                    in0=racc[mt][:], in1=bc_sb[:], op=ALU.mult)

    # --- output projection (partial over local heads), transposed out ---
    for mt in range(2):
        for ch in range(NCH):
            ps = psA.tile([128, SC], dt.float32, tag="mm")
            for g in range(8):            # g = h*2 + kt2 over local hDe tiles
                h, kt2 = g // 2, g % 2
                nc.tensor.matmul(
                    ps[:],
                    wo[:, g * 256 + mt * 128: g * 256 + mt * 128 + 128],
                    rc[(h, kt2)][:, ch * SC: ch * SC + SC],
                    start=(g == 0), stop=(g == 7))
            nc.any.tensor_copy(
                ret_out[:, mt * S + ch * SC: mt * S + ch * SC + SC], ps[:])


def build_nc():
    nc = bacc.Bacc(None, target_bir_lowering=False)

    inp = nc.dram_tensor("inp", [S, 3], dt.int32, kind="ExternalInput")
    qmat = nc.dram_tensor("qmat", [P_TAB, C], dt.float32, kind="ExternalInput")
    ce = nc.dram_tensor("ce", [C, D], dt.float32, kind="ExternalInput")
    de = nc.dram_tensor("de", [C, D], dt.float32, kind="ExternalInput")
    fe = nc.dram_tensor("fe", [C, D], dt.float32, kind="ExternalInput")
    mu = nc.dram_tensor("mu", [C, 1], dt.float32, kind="ExternalInput")
    re = nc.dram_tensor("re", [2, D], dt.float32, kind="ExternalInput")
    dwv = nc.dram_tensor("dwv", [2 * D, 1], dt.float32, kind="ExternalInput")
    dbv = nc.dram_tensor("dbv", [1, 1], dt.float32, kind="ExternalInput")
    ident = nc.dram_tensor("ident", [128, 128], dt.float32, kind="ExternalInput")
    mask_i_x = nc.dram_tensor("mask_i", [128, 128], dt.float32, kind="ExternalInput")
    mask_s_x = nc.dram_tensor("mask_s", [128, 128], dt.float32, kind="ExternalInput")
    wx = {}
    for pre in ("qe", "ke", "kr"):
        for n in ("wq", "wk", "wv"):
            wx[f"{pre}_{n}"] = nc.dram_tensor(
                f"{pre}_{n}", [HL, D, D], dt.float32, kind="ExternalInput")
        wx[f"{pre}_wo"] = nc.dram_tensor(
            f"{pre}_wo", [HL * D, D], dt.float32, kind="ExternalInput")
    out_ext = nc.dram_tensor("out", [1, S], dt.float32, kind="ExternalOutput")

    groups = [[0, 1], [2, 3], [4, 5], [6, 7]]

    from contextlib import ExitStack
    with tile.TileContext(nc) as tc, ExitStack() as es:
        const = es.enter_context(tc.tile_pool(name="const", bufs=1))
        wpool = es.enter_context(tc.tile_pool(name="wpool", bufs=1))
        stage = es.enter_context(tc.tile_pool(name="stage", bufs=2))
        act = es.enter_context(tc.tile_pool(name="act", bufs=1))
        work = es.enter_context(tc.tile_pool(name="work", bufs=2))
        ptp = es.enter_context(tc.tile_pool(name="ptp", bufs=4))
        psA = es.enter_context(tc.tile_pool(name="psA", bufs=3, space="PSUM"))
        psAcc = es.enter_context(tc.tile_pool(name="psAcc", bufs=2, space="PSUM"))
        psRow = es.enter_context(tc.tile_pool(name="psRow", bufs=2, space="PSUM"))
        dram = es.enter_context(tc.tile_pool(name="dram", bufs=1, space="DRAM"))
        pools = {"work": work, "pt": ptp, "psA": psA, "psAcc": psAcc,
                 "psRow": psRow}

        # ---------- constants ----------
        ident_sb = const.tile([128, 128], dt.float32)
        nc.sync.dma_start(ident_sb[:], ident[:])
        ones_col = const.tile([128, 1], dt.bfloat16)
        nc.vector.memset(ones_col[:], 1.0)
        ones_row = const.tile([1, SC], dt.bfloat16)
        nc.vector.memset(ones_row[:], 1.0)
        mask_i_sb = const.tile([128, 128], dt.bfloat16)
        mask_s_sb = const.tile([128, 128], dt.bfloat16)
        for m_sb, m_x in ((mask_i_sb, mask_i_x), (mask_s_sb, mask_s_x)):
            mst = stage.tile([128, 128], dt.float32, tag="mstage", bufs=1)
            nc.sync.dma_start(mst[:], m_x[:])
            nc.vector.tensor_copy(m_sb[:], mst[:])
        consts = {"ones_col": ones_col, "ones_row": ones_row,
                  "mask_i": mask_i_sb, "mask_s": mask_s_sb}

        # ---------- weights -> SBUF bf16 (per-MHA, shared tags) ----------
        def load_w(pre):
            w = {}
            for n in ("wq", "wk", "wv"):
                st = stage.tile([128, HL * 2 * 256], dt.float32, tag="wstage",
                                name=f"wst_{pre}_{n}")
                nc.sync.dma_start(
                    st[:].rearrange("p (h kt e) -> p h kt e", h=HL, kt=2),
                    wx[f"{pre}_{n}"][:].rearrange(
                        "h (kt p) e -> p h kt e", p=128))
                wb = wpool.tile([128, HL * 2 * 256], dt.bfloat16,
                                tag=n, name=f"wb_{pre}_{n}", bufs=2)
                nc.vector.tensor_copy(wb[:], st[:])
                w[n] = wb
            st = stage.tile([128, 8 * 256], dt.float32, tag="wstage",
                            name=f"wst_{pre}_wo")
            nc.sync.dma_start(
                st[:, :8 * 256].rearrange("p (g d) -> p g d", g=8),
                wx[f"{pre}_wo"][:].rearrange("(g p) d -> p g d", p=128))
            wb = wpool.tile([128, 8 * 256], dt.bfloat16, tag="wo",
                            name=f"wb_{pre}_wo", bufs=2)
            nc.vector.tensor_copy(wb[:], st[:, :8 * 256])
            w["wo"] = wb
            return w

        # ---------- embeds: ce2 = c + mu*d, fe2 = mu*f ----------
        de_sb = stage.tile([128, 2 * 256], dt.float32, tag="emb_d", bufs=1)
        ce_sb = stage.tile([128, 2 * 256], dt.float32, tag="emb_c", bufs=1)
        fe_sb = stage.tile([128, 2 * 256], dt.float32, tag="emb_f", bufs=1)
        mu_sb = const.tile([128, 2], dt.float32)
        for t_sb, t_x in ((de_sb, de), (ce_sb, ce), (fe_sb, fe)):
            nc.sync.dma_start(
                t_sb[:].rearrange("p (kt d) -> p kt d", kt=2),
                t_x[:].rearrange("(kt p) d -> p kt d", p=128))
        nc.sync.dma_start(
            mu_sb[:].rearrange("p (kt o) -> p kt o", kt=2),
            mu[:].rearrange("(kt p) o -> p kt o", p=128))
        ce2 = const.tile([128, 2 * 256], dt.bfloat16)
        fe2 = const.tile([128, 2 * 256], dt.bfloat16)
        tmp_md = stage.tile([128, 2 * 256], dt.float32, tag="emb_t", bufs=1)
        for kt in range(2):
            sl = slice(kt * 256, kt * 256 + 256)
            nc.vector.tensor_scalar_mul(tmp_md[:, sl], de_sb[:, sl],
                                        mu_sb[:, kt:kt + 1])
            nc.vector.tensor_tensor(out=ce2[:, sl], in0=tmp_md[:, sl],
                                    in1=ce_sb[:, sl], op=ALU.add)
            nc.vector.tensor_scalar_mul(tmp_md[:, sl], fe_sb[:, sl],
                                        mu_sb[:, kt:kt + 1])
            nc.vector.tensor_copy(fe2[:, sl], tmp_md[:, sl])

        # r_embed rows, d_W, d_b
        r0f = stage.tile([1, D], dt.float32, tag="r0f", bufs=1)
        r1f = stage.tile([1, D], dt.float32, tag="r1f", bufs=1)
        nc.sync.dma_start(r0f[:], re[0:1, :])
        nc.sync.dma_start(r1f[:], re[1:2, :])
        r0b = const.tile([1, D], dt.bfloat16)
        drb = const.tile([1, D], dt.bfloat16)
        nc.vector.tensor_copy(r0b[:], r0f[:])
        nc.vector.tensor_tensor(out=drb[:], in0=r1f[:], in1=r0f[:],
                                op=ALU.subtract)
        dw_f = stage.tile([128, 4], dt.float32, tag="dwf", bufs=1)
        nc.sync.dma_start(
            dw_f[:].rearrange("p (i o) -> p i o", i=4),
            dwv[:].rearrange("(i p) o -> p i o", p=128))
        dw_b = const.tile([128, 4], dt.bfloat16)
        nc.vector.tensor_copy(dw_b[:], dw_f[:])
        db_sb = const.tile([1, 1], dt.float32)
        nc.sync.dma_start(db_sb[:], dbv[:])

        # ---------- gather + transpose concept ----------
        concept_T = act.tile([128, 2 * S], dt.bfloat16, tag="conceptT")
        for t in range(NT):
            idx = stage.tile([128, 1], dt.int32, tag="idx")
            nc.sync.dma_start(idx[:], inp[t * 128:(t + 1) * 128, 0:1])
            idx0 = stage.tile([128, 1], dt.int32, tag="idx0")
            nc.vector.tensor_scalar_add(idx0[:], idx[:], -1)
            cn = stage.tile([128, C], dt.float32, tag="cnat")
            nc.gpsimd.indirect_dma_start(
                out=cn[:], out_offset=None, in_=qmat[:],
                in_offset=bass.IndirectOffsetOnAxis(ap=idx0[:, :1], axis=0))
            for kt in range(2):
                pt_ps = psA.tile([128, SC], dt.float32, tag="mm")
                nc.tensor.transpose(pt_ps[:, :128],
                                    cn[:, kt * 128:(kt + 1) * 128],
                                    ident_sb[:])
                nc.any.tensor_copy(
                    concept_T[:, kt * S + t * 128: kt * S + t * 128 + 128],
                    pt_ps[:, :128])

        # cnum row + corr row
        cnum_f = act.tile([1, S], dt.float32, tag="cnum")
        for ch in range(NCH):
            psr = psRow.tile([1, SC], dt.float32, tag="row")
            for kt in range(2):
                nc.tensor.matmul(
                    psr[:], ones_col[:],
                    concept_T[:, kt * S + ch * SC: kt * S + ch * SC + SC],
                    start=(kt == 0), stop=(kt == 1))
            nc.vector.tensor_copy(cnum_f[:, ch * SC: ch * SC + SC], psr[:])
        corr_i = stage.tile([1, S], dt.int32, tag="corri", bufs=1)
        nc.sync.dma_start(corr_i[:], inp[:].rearrange("s c -> c s")[2:3, :])
        corr_f = stage.tile([1, S], dt.float32, tag="corrf", bufs=1)
        nc.vector.tensor_copy(corr_f[:], corr_i[:])
        s1b = act.tile([1, S], dt.bfloat16, tag="s1b")
        s2b = act.tile([1, S], dt.bfloat16, tag="s2b")
        nc.vector.tensor_copy(s1b[:], cnum_f[:])
        s2f = stage.tile([1, S], dt.float32, tag="s2f", bufs=1)
        nc.vector.tensor_tensor(out=s2f[:], in0=cnum_f[:], in1=corr_f[:],
                                op=ALU.mult)
        nc.vector.tensor_copy(s2b[:], s2f[:])

        # ---------- x_T, y_T ----------
        x_T = act.tile([128, 2 * S], dt.bfloat16, tag="xT")
        y_T = act.tile([128, 2 * S], dt.bfloat16, tag="yT")
        for mt in range(2):
            for ch in range(NCH):
                ps = psA.tile([128, SC], dt.float32, tag="mm")
                for kt in range(2):
                    nc.tensor.matmul(
                        ps[:],
                        ce2[:, kt * 256 + mt * 128: kt * 256 + mt * 128 + 128],
                        concept_T[:, kt * S + ch * SC: kt * S + ch * SC + SC],
                        start=(kt == 0), stop=(kt == 1))
                nc.any.tensor_copy(
                    x_T[:, mt * S + ch * SC: mt * S + ch * SC + SC], ps[:])
                ps2 = psA.tile([128, SC], dt.float32, tag="mm")
                for kt in range(2):
                    nc.tensor.matmul(
                        ps2[:],
                        fe2[:, kt * 256 + mt * 128: kt * 256 + mt * 128 + 128],
                        concept_T[:, kt * S + ch * SC: kt * S + ch * SC + SC],
                        start=(kt == 0), stop=False)
                nc.tensor.matmul(
                    ps2[:], r0b[0:1, mt * 128: mt * 128 + 128],
                    s1b[0:1, ch * SC: ch * SC + SC], start=False, stop=False)
                nc.tensor.matmul(
                    ps2[:], drb[0:1, mt * 128: mt * 128 + 128],
                    s2b[0:1, ch * SC: ch * SC + SC], start=False, stop=True)
                nc.any.tensor_copy(
                    y_T[:, mt * S + ch * SC: mt * S + ch * SC + SC], ps2[:])

        # ---------- qe / ke encoders + pair AllReduce ----------
        xhat_T = act.tile([128, 2 * S], dt.bfloat16, tag="xhatT")
        yhat_T = act.tile([128, 2 * S], dt.bfloat16, tag="yhatT")
        for (src, wpre, hat) in ((x_T, "qe", xhat_T), (y_T, "ke", yhat_T)):
            w = load_w(wpre)
            part = stage.tile([128, 2 * S], dt.bfloat16, tag="part")
            _mha(nc, tc, pools, consts, src, src,
                 w["wq"], w["wk"], w["wv"], w["wo"],
                 strict=False, ret_out=part)
            b_in = dram.tile([128, 2 * S], dt.bfloat16, tag=f"bin_{wpre}")
            b_out = dram.tile([128, 2 * S], dt.bfloat16, tag=f"bout_{wpre}")
            nc.sync.dma_start(b_in[:], part[:])
            nc.gpsimd.collective_compute(
                "AllReduce", ALU.add, replica_groups=groups,
                ins=[b_in[:].opt()], outs=[b_out[:].opt()])
            nc.sync.dma_start(hat[:], b_out[:])

        # ---------- kr ----------
        w = load_w("kr")
        okr = stage.tile([128, 2 * S], dt.bfloat16, tag="part")
        _mha(nc, tc, pools, consts, xhat_T, yhat_T,
             w["wq"], w["wk"], w["wv"], w["wo"],
             strict=True, ret_out=okr)

        # ---------- final head: t1 (AllReduced) + t2, sigmoid ----------
        t1f = stage.tile([1, S], dt.float32, tag="t1f", bufs=1)
        for ch in range(NCH):
            psr = psRow.tile([1, SC], dt.float32, tag="row")
            for kt in range(2):
                nc.tensor.matmul(
                    psr[:], dw_b[:, kt:kt + 1],
                    okr[:, kt * S + ch * SC: kt * S + ch * SC + SC],
                    start=(kt == 0), stop=(kt == 1))
            nc.vector.tensor_copy(t1f[:, ch * SC: ch * SC + SC], psr[:])
        tb_in = dram.tile([1, S], dt.float32, tag="bin_t1")
        tb_out = dram.tile([1, S], dt.float32, tag="bout_t1")
        nc.sync.dma_start(tb_in[:], t1f[:])
        nc.gpsimd.collective_compute(
            "AllReduce", ALU.add, replica_groups=groups,
            ins=[tb_in[:].opt()], outs=[tb_out[:].opt()])
        t1full = stage.tile([1, S], dt.float32, tag="t1full", bufs=1)
        nc.sync.dma_start(t1full[:], tb_out[:])

        pred = stage.tile([1, S], dt.float32, tag="pred", bufs=1)
        for ch in range(NCH):
            psr = psRow.tile([1, SC], dt.float32, tag="row")
            for kt in range(2):
                nc.tensor.matmul(
                    psr[:], dw_b[:, 2 + kt: 3 + kt],
                    xhat_T[:, kt * S + ch * SC: kt * S + ch * SC + SC],
                    start=(kt == 0), stop=(kt == 1))
            ssum = stage.tile([1, SC], dt.float32, tag="ssum", bufs=2)
            nc.vector.tensor_tensor(
                out=ssum[:], in0=t1full[:, ch * SC: ch * SC + SC], in1=psr[:],
                op=ALU.add)
            nc.scalar.activation(pred[:, ch * SC: ch * SC + SC], ssum[:],
                                 AF.Sigmoid, bias=db_sb[0:1, 0:1])
        nc.sync.dma_start(out_ext[:], pred[:])

    nc.finalize()
    return nc


_NC_CACHE = None


def _get_nc():
    global _NC_CACHE
    if _NC_CACHE is None:
        _NC_CACHE = build_nc()
    return _NC_CACHE


def make_in_maps(inputs):
    f32 = np.float32
    common = {
        "qmat": np.ascontiguousarray(np.asarray(inputs["Q_matrix"], f32)),
        "ce": np.asarray(inputs["c_embed"], f32),
        "de": np.asarray(inputs["d_embed"], f32),
        "fe": np.asarray(inputs["f_embed"], f32),
        "mu": np.asarray(inputs["mu_q"], f32),
        "re": np.asarray(inputs["r_embed"], f32),
        "dwv": np.asarray(inputs["d_W"], f32),
        "dbv": np.asarray(inputs["d_b"], f32).reshape(1, 1),
        "ident": np.eye(128, dtype=f32),
        "mask_i": np.triu(np.ones((128, 128), f32), 0),
        "mask_s": np.triu(np.ones((128, 128), f32), 1),
    }
    inp_all = np.asarray(inputs["inputs"], np.int32)
    in_maps = []
    for c in range(8):
        b, h0 = c // 2, (c % 2) * HL
        m = dict(common)
        m["inp"] = np.ascontiguousarray(inp_all[b])
        for pre in ("qe", "ke", "kr"):
            m[f"{pre}_wq"] = np.ascontiguousarray(
                np.asarray(inputs[f"{pre}_wQ"], f32)[h0:h0 + HL])
            m[f"{pre}_wk"] = np.ascontiguousarray(
                np.asarray(inputs[f"{pre}_wK"], f32)[h0:h0 + HL])
            m[f"{pre}_wv"] = np.ascontiguousarray(
                np.asarray(inputs[f"{pre}_wV"], f32)[h0:h0 + HL])
            m[f"{pre}_wo"] = np.ascontiguousarray(
                np.asarray(inputs[f"{pre}_wO"], f32)[h0 * D:(h0 + HL) * D])
        in_maps.append(m)
    return in_maps


def kernel(**inputs):
    nc = _get_nc()
    in_maps = make_in_maps(inputs)
    res = run_bass_kernel_spmd(nc, in_maps, core_ids=list(range(8)))
    outs = res.results
    pred = np.stack([outs[2 * b]["out"].reshape(S) for b in range(B)])
    return pred[..., None].astype(np.float32)


# revision 19
# speedup vs baseline: 1.0123x; 1.0123x over previous
"""AKT (sparse attention) Trainium2 kernel — 8 NeuronCores.

Sharding: data-parallel over batch B=4 (pairs of cores share a batch) x
tensor-parallel over heads (each core owns 4 of the 8 heads). Attention
output projections produce per-core partial sums that are AllReduced
within each pair of cores; the final prediction row is AllReduced the
same way (tiny, [1,S]).

Math notes:
  - reference does softmax -> tril mask -> renormalize, which is exactly a
    masked (causal) softmax, so we only compute the lower-triangular blocks.
  - scores are tiny (|s| << 1), so no max-subtraction is needed for exp.
  - kr (strictly causal) row 0 has empty support; den + eps keeps it a clean
    zero, and the downstream shift discards that row anyway.

All matmuls run in bf16 with f32 PSUM accumulation (rel tolerance 2e-2;
values here are tiny and well-conditioned, measured rel err ~1e-4).
"""

import sys

if "/opt/trn_rl_repo" not in sys.path:
    sys.path.insert(0, "/opt/trn_rl_repo")

import numpy as np

import concourse.bass as bass
import concourse.bacc as bacc
import concourse.tile as tile
import concourse.mybir as mybir
from concourse.bass_utils import run_bass_kernel_spmd

dt = mybir.dt
AF = mybir.ActivationFunctionType
ALU = mybir.AluOpType

B, S, D, H = 4, 1024, 256, 8
P_TAB, C = 10000, 256
HL = H // 2          # heads per core
NT = S // 128        # 8 sequence tiles of 128
SC = 512             # free-dim chunk (one PSUM bank of f32)
NCH = S // SC        # 2 chunks


def _mha(nc, tc, pools, consts, qk_src, v_src, wq, wk, wv, wo, strict, ret_out):
    """One multi-head attention on 4 local heads.

    qk_src / v_src: SBUF bf16 [128, 2*S]  (transposed activations, free = kt*S + s)
    wq/wk/wv: SBUF bf16 [128, HL*2*256]   (free = h*512 + kt*256 + e)
    wo:       SBUF bf16 [128, 8*256]      (free = g*256 + d, g = local hDe tile)
    ret_out:  SBUF bf16 [128, 2*S]        (normalized+projected partial out, transposed)
    """
    work, ptp, psA, psAcc, psRow = (
        pools["work"], pools["pt"], pools["psA"], pools["psAcc"], pools["psRow"])
    ones_col, ones_row, mask = consts["ones_col"], consts["ones_row"], (
        consts["mask_s"] if strict else consts["mask_i"])

    rc = {}
    for h in range(HL):
        # --- projections ---
        qT = work.tile([128, 2 * S], dt.bfloat16, tag="qT")
        kT = work.tile([128, 2 * S], dt.bfloat16, tag="kT")
        for dst, w_sb in ((qT, wq), (kT, wk)):
            for mt in range(2):           # De half (output partition tile)
                for ch in range(NCH):
                    ps = psA.tile([128, SC], dt.float32, tag="mm")
                    for kt in range(2):   # D (contraction) half
                        nc.tensor.matmul(
                            ps[:],
                            w_sb[:, h * 512 + kt * 256 + mt * 128:
                                 h * 512 + kt * 256 + mt * 128 + 128],
                            qk_src[:, kt * S + ch * SC: kt * S + ch * SC + SC],
                            start=(kt == 0), stop=(kt == 1))
                    nc.any.tensor_copy(
                        dst[:, mt * S + ch * SC: mt * S + ch * SC + SC], ps[:])
        vN = work.tile([128, NT * 256], dt.bfloat16, tag="vN")
        for sp in range(NT // 2):         # V natural: [s-tile partitions, De free]
            ps = psA.tile([128, SC], dt.float32, tag="mm")
            for half in range(2):         # two s-tiles share one PSUM bank
                st = sp * 2 + half
                for kt in range(2):
                    nc.tensor.matmul(
                        ps[:, half * 256: half * 256 + 256],
                        v_src[:, kt * S + st * 128: kt * S + st * 128 + 128],
                        wv[:, h * 512 + kt * 256: h * 512 + kt * 256 + 256],
                        start=(kt == 0), stop=(kt == 1))
            nc.any.tensor_copy(vN[:, sp * 512: sp * 512 + 512], ps[:])

        # --- causal attention, transposed-softmax ---
        for mt in range(2):
            rc[(h, mt)] = work.tile([128, S], dt.bfloat16, tag=f"rc{h}{mt}",
                                    name=f"rc{h}{mt}", bufs=1)
        for qc in range(NCH):
            racc = [psAcc.tile([128, SC], dt.float32, tag="racc", name=f"racc{i}") for i in range(2)]
            dacc = psRow.tile([1, SC], dt.float32, tag="row")
            nk = 4 * (qc + 1)
            for kb in range(nk):
                n_off = max(0, kb * 128 - qc * SC)
                nq = SC - n_off
                q0 = qc * SC + n_off
                ps_s = psA.tile([128, SC], dt.float32, tag="mm")
                for kt in range(2):
                    nc.tensor.matmul(
                        ps_s[:, :nq],
                        kT[:, kt * S + kb * 128: kt * S + kb * 128 + 128],
                        qT[:, kt * S + q0: kt * S + q0 + nq],
                        start=(kt == 0), stop=(kt == 1))
                pt = ptp.tile([128, SC], dt.bfloat16, tag="pt")
                # exp(s/sqrt(D)); bf16 out for the AV matmul
                nc.scalar.activation(pt[:, :nq], ps_s[:, :nq], AF.Exp,
                                     scale=1.0 / 16.0)
                if kb >= qc * 4:  # diagonal 128-block is the first 128 cols
                    nc.vector.tensor_tensor(
                        out=pt[:, :128], in0=pt[:, :128], in1=mask[:],
                        op=ALU.mult)
                for mt in range(2):
                    nc.tensor.matmul(
                        racc[mt][:, n_off:SC],
                        vN[:, kb * 256 + mt * 128: kb * 256 + mt * 128 + 128],
                        pt[:, :nq],
                        start=(kb == 0), stop=(kb == nk - 1))
                nc.tensor.matmul(
                    dacc[:, n_off:SC], ones_col[:], pt[:, :nq],
                    start=(kb == 0), stop=(kb == nk - 1))
            # free the PSUM accumulators right away (keeps next k-loop's AV
            # matmuls unblocked), then normalize from SBUF
            rcu = [ptp.tile([128, SC], dt.float32, tag="rcu", bufs=2,
                            name=f"rcu{i}") for i in range(2)]
            for mt in range(2):
                nc.vector.tensor_copy(rcu[mt][:], racc[mt][:])
            rec_f = ptp.tile([1, SC], dt.float32, tag="rec_f", bufs=2)
            if strict:
                den = ptp.tile([1, SC], dt.float32, tag="den", bufs=2)
                nc.vector.tensor_scalar_add(den[:], dacc[:], 1e-20)
                nc.vector.reciprocal_approx_fast(rec_f[:], den[:])
            else:
                nc.vector.reciprocal_approx_fast(rec_f[:], dacc[:])
            bc_sb = ptp.tile([128, SC], dt.float32, tag="bcs", bufs=2)
            nc.gpsimd.partition_broadcast(bc_sb[:], rec_f[:])
            for mt in range(2):
                nc.vector.tensor_tensor(
                    out=rc[(h, mt)][:, qc * SC: qc * SC + SC],
                    in0=rcu[mt][:], in1=bc_sb[:], op=ALU.mult)

    # --- output projection (partial over local heads), transposed out ---
    for mt in range(2):
        for ch in range(NCH):
            ps = psA.tile([128, SC], dt.float32, tag="mm")
            for g in range(8):            # g = h*2 + kt2 over local hDe tiles
                h, kt2 = g // 2, g % 2
                nc.tensor.matmul(
                    ps[:],
                    wo[:, g * 256 + mt * 128: g * 256 + mt * 128 + 128],
                    rc[(h, kt2)][:, ch * SC: ch * SC + SC],
                    start=(g == 0), stop=(g == 7))
            nc.any.tensor_copy(
                ret_out[:, mt * S + ch * SC: mt * S + ch * SC + SC], ps[:])


def build_nc():
    nc = bacc.Bacc(None, target_bir_lowering=False)

    inp = nc.dram_tensor("inp", [S, 3], dt.int32, kind="ExternalInput")
    qmat = nc.dram_tensor("qmat", [P_TAB, C], dt.float32, kind="ExternalInput")
    ce = nc.dram_tensor("ce", [C, D], dt.float32, kind="ExternalInput")
    de = nc.dram_tensor("de", [C, D], dt.float32, kind="ExternalInput")
    fe = nc.dram_tensor("fe", [C, D], dt.float32, kind="ExternalInput")
    mu = nc.dram_tensor("mu", [C, 1], dt.float32, kind="ExternalInput")
    re = nc.dram_tensor("re", [2, D], dt.float32, kind="ExternalInput")
    dwv = nc.dram_tensor("dwv", [2 * D, 1], dt.float32, kind="ExternalInput")
    dbv = nc.dram_tensor("dbv", [1, 1], dt.float32, kind="ExternalInput")
    ident = nc.dram_tensor("ident", [128, 128], dt.float32, kind="ExternalInput")
    mask_i_x = nc.dram_tensor("mask_i", [128, 128], dt.float32, kind="ExternalInput")
    mask_s_x = nc.dram_tensor("mask_s", [128, 128], dt.float32, kind="ExternalInput")
    wx = {}
    for pre in ("qe", "ke", "kr"):
        for n in ("wq", "wk", "wv"):
            wx[f"{pre}_{n}"] = nc.dram_tensor(
                f"{pre}_{n}", [HL, D, D], dt.float32, kind="ExternalInput")
        wx[f"{pre}_wo"] = nc.dram_tensor(
            f"{pre}_wo", [HL * D, D], dt.float32, kind="ExternalInput")
    out_ext = nc.dram_tensor("out", [1, S], dt.float32, kind="ExternalOutput")

    groups = [[0, 1], [2, 3], [4, 5], [6, 7]]

    from contextlib import ExitStack
    with tile.TileContext(nc) as tc, ExitStack() as es:
        const = es.enter_context(tc.tile_pool(name="const", bufs=1))
        wpool = es.enter_context(tc.tile_pool(name="wpool", bufs=1))
        stage = es.enter_context(tc.tile_pool(name="stage", bufs=2))
        act = es.enter_context(tc.tile_pool(name="act", bufs=1))
        work = es.enter_context(tc.tile_pool(name="work", bufs=2))
        ptp = es.enter_context(tc.tile_pool(name="ptp", bufs=4))
        psA = es.enter_context(tc.tile_pool(name="psA", bufs=2, space="PSUM"))
        psAcc = es.enter_context(tc.tile_pool(name="psAcc", bufs=4, space="PSUM"))
        psRow = es.enter_context(tc.tile_pool(name="psRow", bufs=2, space="PSUM"))
        dram = es.enter_context(tc.tile_pool(name="dram", bufs=1, space="DRAM"))
        pools = {"work": work, "pt": ptp, "psA": psA, "psAcc": psAcc,
                 "psRow": psRow}

        # ---------- constants ----------
        ident_sb = const.tile([128, 128], dt.float32)
        nc.sync.dma_start(ident_sb[:], ident[:])
        ones_col = const.tile([128, 1], dt.bfloat16)
        nc.vector.memset(ones_col[:], 1.0)
        ones_row = const.tile([1, SC], dt.bfloat16)
        nc.vector.memset(ones_row[:], 1.0)
        mask_i_sb = const.tile([128, 128], dt.bfloat16)
        mask_s_sb = const.tile([128, 128], dt.bfloat16)
        for m_sb, m_x in ((mask_i_sb, mask_i_x), (mask_s_sb, mask_s_x)):
            mst = stage.tile([128, 128], dt.float32, tag="mstage", bufs=1)
            nc.sync.dma_start(mst[:], m_x[:])
            nc.vector.tensor_copy(m_sb[:], mst[:])
        consts = {"ones_col": ones_col, "ones_row": ones_row,
                  "mask_i": mask_i_sb, "mask_s": mask_s_sb}

        # ---------- weights -> SBUF bf16 (per-MHA, shared tags) ----------
        def load_w(pre):
            w = {}
            for n in ("wq", "wk", "wv"):
                st = stage.tile([128, HL * 2 * 256], dt.float32, tag="wstage",
                                name=f"wst_{pre}_{n}")
                nc.sync.dma_start(
                    st[:].rearrange("p (h kt e) -> p h kt e", h=HL, kt=2),
                    wx[f"{pre}_{n}"][:].rearrange(
                        "h (kt p) e -> p h kt e", p=128))
                wb = wpool.tile([128, HL * 2 * 256], dt.bfloat16,
                                tag=n, name=f"wb_{pre}_{n}", bufs=2)
                nc.gpsimd.tensor_copy(wb[:], st[:])
                w[n] = wb
            st = stage.tile([128, 8 * 256], dt.float32, tag="wstage",
                            name=f"wst_{pre}_wo")
            nc.sync.dma_start(
                st[:, :8 * 256].rearrange("p (g d) -> p g d", g=8),
                wx[f"{pre}_wo"][:].rearrange("(g p) d -> p g d", p=128))
            wb = wpool.tile([128, 8 * 256], dt.bfloat16, tag="wo",
                            name=f"wb_{pre}_wo", bufs=2)
            nc.gpsimd.tensor_copy(wb[:], st[:, :8 * 256])
            w["wo"] = wb
            return w

        # ---------- embeds: ce2 = c + mu*d, fe2 = mu*f ----------
        de_sb = stage.tile([128, 2 * 256], dt.float32, tag="emb_d", bufs=1)
        ce_sb = stage.tile([128, 2 * 256], dt.float32, tag="emb_c", bufs=1)
        fe_sb = stage.tile([128, 2 * 256], dt.float32, tag="emb_f", bufs=1)
        mu_sb = const.tile([128, 2], dt.float32)
        for t_sb, t_x in ((de_sb, de), (ce_sb, ce), (fe_sb, fe)):
            nc.sync.dma_start(
                t_sb[:].rearrange("p (kt d) -> p kt d", kt=2),
                t_x[:].rearrange("(kt p) d -> p kt d", p=128))
        nc.sync.dma_start(
            mu_sb[:].rearrange("p (kt o) -> p kt o", kt=2),
            mu[:].rearrange("(kt p) o -> p kt o", p=128))
        ce2 = const.tile([128, 2 * 256], dt.bfloat16)
        fe2 = const.tile([128, 2 * 256], dt.bfloat16)
        tmp_md = stage.tile([128, 2 * 256], dt.float32, tag="emb_t", bufs=1)
        for kt in range(2):
            sl = slice(kt * 256, kt * 256 + 256)
            nc.vector.tensor_scalar_mul(tmp_md[:, sl], de_sb[:, sl],
                                        mu_sb[:, kt:kt + 1])
            nc.vector.tensor_tensor(out=ce2[:, sl], in0=tmp_md[:, sl],
                                    in1=ce_sb[:, sl], op=ALU.add)
            nc.vector.tensor_scalar_mul(tmp_md[:, sl], fe_sb[:, sl],
                                        mu_sb[:, kt:kt + 1])
            nc.vector.tensor_copy(fe2[:, sl], tmp_md[:, sl])

        # r_embed rows, d_W, d_b
        r0f = stage.tile([1, D], dt.float32, tag="r0f", bufs=1)
        r1f = stage.tile([1, D], dt.float32, tag="r1f", bufs=1)
        nc.sync.dma_start(r0f[:], re[0:1, :])
        nc.sync.dma_start(r1f[:], re[1:2, :])
        r0b = const.tile([1, D], dt.bfloat16)
        drb = const.tile([1, D], dt.bfloat16)
        nc.vector.tensor_copy(r0b[:], r0f[:])
        nc.vector.tensor_tensor(out=drb[:], in0=r1f[:], in1=r0f[:],
                                op=ALU.subtract)
        dw_f = stage.tile([128, 4], dt.float32, tag="dwf", bufs=1)
        nc.sync.dma_start(
            dw_f[:].rearrange("p (i o) -> p i o", i=4),
            dwv[:].rearrange("(i p) o -> p i o", p=128))
        dw_b = const.tile([128, 4], dt.bfloat16)
        nc.vector.tensor_copy(dw_b[:], dw_f[:])
        db_sb = const.tile([1, 1], dt.float32)
        nc.sync.dma_start(db_sb[:], dbv[:])

        # ---------- gather + transpose concept ----------
        concept_T = act.tile([128, 2 * S], dt.bfloat16, tag="conceptT")
        for t in range(NT):
            idx = stage.tile([128, 1], dt.int32, tag="idx")
            nc.sync.dma_start(idx[:], inp[t * 128:(t + 1) * 128, 0:1])
            idx0 = stage.tile([128, 1], dt.int32, tag="idx0")
            nc.vector.tensor_scalar_add(idx0[:], idx[:], -1)
            cn = stage.tile([128, C], dt.float32, tag="cnat")
            nc.gpsimd.indirect_dma_start(
                out=cn[:], out_offset=None, in_=qmat[:],
                in_offset=bass.IndirectOffsetOnAxis(ap=idx0[:, :1], axis=0))
            for kt in range(2):
                pt_ps = psA.tile([128, SC], dt.float32, tag="mm")
                nc.tensor.transpose(pt_ps[:, :128],
                                    cn[:, kt * 128:(kt + 1) * 128],
                                    ident_sb[:])
                nc.any.tensor_copy(
                    concept_T[:, kt * S + t * 128: kt * S + t * 128 + 128],
                    pt_ps[:, :128])

        # cnum row + corr row
        cnum_f = act.tile([1, S], dt.float32, tag="cnum")
        for ch in range(NCH):
            psr = psA.tile([1, SC], dt.float32, tag="mm", name="psr_cnum")
            for kt in range(2):
                nc.tensor.matmul(
                    psr[:], ones_col[:],
                    concept_T[:, kt * S + ch * SC: kt * S + ch * SC + SC],
                    start=(kt == 0), stop=(kt == 1))
            nc.vector.tensor_copy(cnum_f[:, ch * SC: ch * SC + SC], psr[:])
        corr_i = stage.tile([1, S], dt.int32, tag="corri", bufs=1)
        nc.sync.dma_start(corr_i[:], inp[:].rearrange("s c -> c s")[2:3, :])
        corr_f = stage.tile([1, S], dt.float32, tag="corrf", bufs=1)
        nc.vector.tensor_copy(corr_f[:], corr_i[:])
        s1b = act.tile([1, S], dt.bfloat16, tag="s1b")
        s2b = act.tile([1, S], dt.bfloat16, tag="s2b")
        nc.vector.tensor_copy(s1b[:], cnum_f[:])
        s2f = stage.tile([1, S], dt.float32, tag="s2f", bufs=1)
        nc.vector.tensor_tensor(out=s2f[:], in0=cnum_f[:], in1=corr_f[:],
                                op=ALU.mult)
        nc.vector.tensor_copy(s2b[:], s2f[:])

        # ---------- x_T, y_T ----------
        x_T = act.tile([128, 2 * S], dt.bfloat16, tag="xT")
        y_T = act.tile([128, 2 * S], dt.bfloat16, tag="yT")
        for mt in range(2):
            for ch in range(NCH):
                ps = psA.tile([128, SC], dt.float32, tag="mm")
                for kt in range(2):
                    nc.tensor.matmul(
                        ps[:],
                        ce2[:, kt * 256 + mt * 128: kt * 256 + mt * 128 + 128],
                        concept_T[:, kt * S + ch * SC: kt * S + ch * SC + SC],
                        start=(kt == 0), stop=(kt == 1))
                nc.any.tensor_copy(
                    x_T[:, mt * S + ch * SC: mt * S + ch * SC + SC], ps[:])
                ps2 = psA.tile([128, SC], dt.float32, tag="mm")
                for kt in range(2):
                    nc.tensor.matmul(
                        ps2[:],
                        fe2[:, kt * 256 + mt * 128: kt * 256 + mt * 128 + 128],
                        concept_T[:, kt * S + ch * SC: kt * S + ch * SC + SC],
                        start=(kt == 0), stop=False)
                nc.tensor.matmul(
                    ps2[:], r0b[0:1, mt * 128: mt * 128 + 128],
                    s1b[0:1, ch * SC: ch * SC + SC], start=False, stop=False)
                nc.tensor.matmul(
                    ps2[:], drb[0:1, mt * 128: mt * 128 + 128],
                    s2b[0:1, ch * SC: ch * SC + SC], start=False, stop=True)
                nc.any.tensor_copy(
                    y_T[:, mt * S + ch * SC: mt * S + ch * SC + SC], ps2[:])

        # ---------- qe / ke encoders + pair AllReduce ----------
        xhat_T = act.tile([128, 2 * S], dt.bfloat16, tag="xhatT")
        yhat_T = act.tile([128, 2 * S], dt.bfloat16, tag="yhatT")
        for (src, wpre, hat) in ((x_T, "qe", xhat_T), (y_T, "ke", yhat_T)):
            w = load_w(wpre)
            part = stage.tile([128, 2 * S], dt.bfloat16, tag="part")
            _mha(nc, tc, pools, consts, src, src,
                 w["wq"], w["wk"], w["wv"], w["wo"],
                 strict=False, ret_out=part)
            b_in = dram.tile([128, 2 * S], dt.bfloat16, tag=f"bin_{wpre}")
            b_out = dram.tile([128, 2 * S], dt.bfloat16, tag=f"bout_{wpre}")
            nc.sync.dma_start(b_in[:], part[:])
            nc.gpsimd.collective_compute(
                "AllReduce", ALU.add, replica_groups=groups,
                ins=[b_in[:].opt()], outs=[b_out[:].opt()])
            nc.sync.dma_start(hat[:], b_out[:])

        # ---------- kr ----------
        w = load_w("kr")
        okr = stage.tile([128, 2 * S], dt.bfloat16, tag="part")
        _mha(nc, tc, pools, consts, xhat_T, yhat_T,
             w["wq"], w["wk"], w["wv"], w["wo"],
             strict=True, ret_out=okr)

        # ---------- final head: t1 (AllReduced) + t2, sigmoid ----------
        t1f = stage.tile([1, S], dt.float32, tag="t1f", bufs=1)
        for ch in range(NCH):
            psr = psA.tile([1, SC], dt.float32, tag="mm", name="psr_t1")
            for kt in range(2):
                nc.tensor.matmul(
                    psr[:], dw_b[:, kt:kt + 1],
                    okr[:, kt * S + ch * SC: kt * S + ch * SC + SC],
                    start=(kt == 0), stop=(kt == 1))
            nc.vector.tensor_copy(t1f[:, ch * SC: ch * SC + SC], psr[:])
        tb_in = dram.tile([1, S], dt.float32, tag="bin_t1")
        tb_out = dram.tile([1, S], dt.float32, tag="bout_t1")
        nc.sync.dma_start(tb_in[:], t1f[:])
        nc.gpsimd.collective_compute(
            "AllReduce", ALU.add, replica_groups=groups,
            ins=[tb_in[:].opt()], outs=[tb_out[:].opt()])
        t1full = stage.tile([1, S], dt.float32, tag="t1full", bufs=1)
        nc.sync.dma_start(t1full[:], tb_out[:])

        pred = stage.tile([1, S], dt.float32, tag="pred", bufs=1)
        for ch in range(NCH):
            psr = psA.tile([1, SC], dt.float32, tag="mm", name="psr_t2")
            for kt in range(2):
                nc.tensor.matmul(
                    psr[:], dw_b[:, 2 + kt: 3 + kt],
                    xhat_T[:, kt * S + ch * SC: kt * S + ch * SC + SC],
                    start=(kt == 0), stop=(kt == 1))
            ssum = stage.tile([1, SC], dt.float32, tag="ssum", bufs=2)
            nc.vector.tensor_tensor(
                out=ssum[:], in0=t1full[:, ch * SC: ch * SC + SC], in1=psr[:],
                op=ALU.add)
            nc.scalar.activation(pred[:, ch * SC: ch * SC + SC], ssum[:],
                                 AF.Sigmoid, bias=db_sb[0:1, 0:1])
        nc.sync.dma_start(out_ext[:], pred[:])

    nc.finalize()
    return nc


_NC_CACHE = None


def _get_nc():
    global _NC_CACHE
    if _NC_CACHE is None:
        _NC_CACHE = build_nc()
    return _NC_CACHE


def make_in_maps(inputs):
    f32 = np.float32
    common = {
        "qmat": np.ascontiguousarray(np.asarray(inputs["Q_matrix"], f32)),
        "ce": np.asarray(inputs["c_embed"], f32),
        "de": np.asarray(inputs["d_embed"], f32),
        "fe": np.asarray(inputs["f_embed"], f32),
        "mu": np.asarray(inputs["mu_q"], f32),
        "re": np.asarray(inputs["r_embed"], f32),
        "dwv": np.asarray(inputs["d_W"], f32),
        "dbv": np.asarray(inputs["d_b"], f32).reshape(1, 1),
        "ident": np.eye(128, dtype=f32),
        "mask_i": np.triu(np.ones((128, 128), f32), 0),
        "mask_s": np.triu(np.ones((128, 128), f32), 1),
    }
    inp_all = np.asarray(inputs["inputs"], np.int32)
    in_maps = []
    for c in range(8):
        b, h0 = c // 2, (c % 2) * HL
        m = dict(common)
        m["inp"] = np.ascontiguousarray(inp_all[b])
        for pre in ("qe", "ke", "kr"):
            m[f"{pre}_wq"] = np.ascontiguousarray(
                np.asarray(inputs[f"{pre}_wQ"], f32)[h0:h0 + HL])
            m[f"{pre}_wk"] = np.ascontiguousarray(
                np.asarray(inputs[f"{pre}_wK"], f32)[h0:h0 + HL])
            m[f"{pre}_wv"] = np.ascontiguousarray(
                np.asarray(inputs[f"{pre}_wV"], f32)[h0:h0 + HL])
            m[f"{pre}_wo"] = np.ascontiguousarray(
                np.asarray(inputs[f"{pre}_wO"], f32)[h0 * D:(h0 + HL) * D])
        in_maps.append(m)
    return in_maps


def kernel(**inputs):
    nc = _get_nc()
    in_maps = make_in_maps(inputs)
    res = run_bass_kernel_spmd(nc, in_maps, core_ids=list(range(8)))
    outs = res.results
    pred = np.stack([outs[2 * b]["out"].reshape(S) for b in range(B)])
    return pred[..., None].astype(np.float32)


# revision 22
# speedup vs baseline: 1.2278x; 1.2129x over previous
"""AKT (sparse attention) Trainium2 kernel — 8 NeuronCores.

Sharding: data-parallel over batch B=4 (pairs of cores share a batch) x
tensor-parallel over heads (each core owns 4 of the 8 heads). Attention
output projections produce per-core partial sums that are AllReduced
within each pair of cores; the final prediction row is AllReduced the
same way (tiny, [1,S]).

Math notes:
  - reference does softmax -> tril mask -> renormalize, which is exactly a
    masked (causal) softmax, so we only compute the lower-triangular blocks.
  - scores are tiny (|s| << 1), so no max-subtraction is needed for exp.
  - kr (strictly causal) row 0 has empty support; den + eps keeps it a clean
    zero, and the downstream shift discards that row anyway.

All matmuls run in bf16 with f32 PSUM accumulation (rel tolerance 2e-2;
values here are tiny and well-conditioned, measured rel err ~1e-4).
"""

import sys

if "/opt/trn_rl_repo" not in sys.path:
    sys.path.insert(0, "/opt/trn_rl_repo")

import numpy as np

import concourse.bass as bass
import concourse.bacc as bacc
import concourse.tile as tile
import concourse.mybir as mybir
from concourse.bass_utils import run_bass_kernel_spmd

dt = mybir.dt
AF = mybir.ActivationFunctionType
ALU = mybir.AluOpType

B, S, D, H = 4, 1024, 256, 8
P_TAB, C = 10000, 256
HL = H // 2          # heads per core
NT = S // 128        # 8 sequence tiles of 128
SC = 512             # free-dim chunk (one PSUM bank of f32)
NCH = S // SC        # 2 chunks


def _mha(nc, tc, pools, consts, qk_src, v_src, wq, wk, wv, wo, strict, ret_out):
    """One multi-head attention on 4 local heads.

    qk_src / v_src: SBUF bf16 [128, 2*S]  (transposed activations, free = kt*S + s)
    wq/wk/wv: SBUF bf16 [128, HL*2*256]   (free = h*512 + kt*256 + e)
    wo:       SBUF bf16 [128, 8*256]      (free = g*256 + d, g = local hDe tile)
    ret_out:  SBUF bf16 [128, 2*S]        (normalized+projected partial out, transposed)
    """
    work, ptp, psA, psAcc, psRow = (
        pools["work"], pools["pt"], pools["psA"], pools["psAcc"], pools["psRow"])
    ones_col, ones_row, mask = consts["ones_col"], consts["ones_row"], (
        consts["mask_s"] if strict else consts["mask_i"])

    rc = {}
    for h in range(HL):
        # --- projections ---
        qT = work.tile([128, 2 * S], dt.bfloat16, tag="qT")
        kT = work.tile([128, 2 * S], dt.bfloat16, tag="kT")
        for dst, w_sb in ((qT, wq), (kT, wk)):
            for mt in range(2):           # De half (output partition tile)
                for ch in range(NCH):
                    ps = psA.tile([128, SC], dt.float32, tag="mm")
                    for kt in range(2):   # D (contraction) half
                        nc.tensor.matmul(
                            ps[:],
                            w_sb[:, h * 512 + kt * 256 + mt * 128:
                                 h * 512 + kt * 256 + mt * 128 + 128],
                            qk_src[:, kt * S + ch * SC: kt * S + ch * SC + SC],
                            start=(kt == 0), stop=(kt == 1))
                    nc.any.tensor_copy(
                        dst[:, mt * S + ch * SC: mt * S + ch * SC + SC], ps[:])
        vN = work.tile([128, NT * 256], dt.bfloat16, tag="vN")
        for sp in range(NT // 2):         # V natural: [s-tile partitions, De free]
            ps = psA.tile([128, SC], dt.float32, tag="mm")
            for half in range(2):         # two s-tiles share one PSUM bank
                st = sp * 2 + half
                for kt in range(2):
                    nc.tensor.matmul(
                        ps[:, half * 256: half * 256 + 256],
                        v_src[:, kt * S + st * 128: kt * S + st * 128 + 128],
                        wv[:, h * 512 + kt * 256: h * 512 + kt * 256 + 256],
                        start=(kt == 0), stop=(kt == 1))
            nc.any.tensor_copy(vN[:, sp * 512: sp * 512 + 512], ps[:])

        # --- causal attention, transposed-softmax ---
        for mt in range(2):
            rc[(h, mt)] = work.tile([128, S], dt.bfloat16, tag=f"rc{h}{mt}",
                                    name=f"rc{h}{mt}", bufs=1)
        for qc in range(NCH):
            racc = [psAcc.tile([128, SC], dt.float32, tag="racc", name=f"racc{i}") for i in range(2)]
            dacc = psRow.tile([1, SC], dt.float32, tag="row")
            nk = 4 * (qc + 1)
            for kb in range(nk):
                n_off = max(0, kb * 128 - qc * SC)
                nq = SC - n_off
                q0 = qc * SC + n_off
                ps_s = psA.tile([128, SC], dt.float32, tag="mm")
                for kt in range(2):
                    nc.tensor.matmul(
                        ps_s[:, :nq],
                        kT[:, kt * S + kb * 128: kt * S + kb * 128 + 128],
                        qT[:, kt * S + q0: kt * S + q0 + nq],
                        start=(kt == 0), stop=(kt == 1))
                pt = ptp.tile([128, SC], dt.bfloat16, tag="pt")
                # exp(s/sqrt(D)); bf16 out for the AV matmul
                nc.scalar.activation(pt[:, :nq], ps_s[:, :nq], AF.Exp,
                                     scale=1.0 / 16.0)
                if kb >= qc * 4:  # diagonal 128-block is the first 128 cols
                    nc.vector.tensor_tensor(
                        out=pt[:, :128], in0=pt[:, :128], in1=mask[:],
                        op=ALU.mult)
                for mt in range(2):
                    nc.tensor.matmul(
                        racc[mt][:, n_off:SC],
                        vN[:, kb * 256 + mt * 128: kb * 256 + mt * 128 + 128],
                        pt[:, :nq],
                        start=(kb == 0), stop=(kb == nk - 1))
                nc.tensor.matmul(
                    dacc[:, n_off:SC], ones_col[:], pt[:, :nq],
                    start=(kb == 0), stop=(kb == nk - 1))
            # free the PSUM accumulators right away (keeps next k-loop's AV
            # matmuls unblocked), then normalize from SBUF
            rcu = [ptp.tile([128, SC], dt.float32, tag="rcu", bufs=2,
                            name=f"rcu{i}") for i in range(2)]
            for mt in range(2):
                nc.vector.tensor_copy(rcu[mt][:], racc[mt][:])
            rec_f = ptp.tile([1, SC], dt.float32, tag="rec_f", bufs=1)
            if strict:
                den = ptp.tile([1, SC], dt.float32, tag="den", bufs=1)
                nc.vector.tensor_scalar_add(den[:], dacc[:], 1e-20)
                nc.vector.reciprocal_approx_fast(rec_f[:], den[:])
            else:
                nc.vector.reciprocal_approx_fast(rec_f[:], dacc[:])
            bc_sb = ptp.tile([128, SC], dt.float32, tag="bcs", bufs=1)
            nc.gpsimd.partition_broadcast(bc_sb[:], rec_f[:])
            for mt in range(2):
                nc.vector.tensor_tensor(
                    out=rc[(h, mt)][:, qc * SC: qc * SC + SC],
                    in0=rcu[mt][:], in1=bc_sb[:], op=ALU.mult)

    # --- output projection (partial over local heads), transposed out ---
    for mt in range(2):
        for ch in range(NCH):
            ps = psA.tile([128, SC], dt.float32, tag="mm")
            for g in range(8):            # g = h*2 + kt2 over local hDe tiles
                h, kt2 = g // 2, g % 2
                nc.tensor.matmul(
                    ps[:],
                    wo[:, g * 256 + mt * 128: g * 256 + mt * 128 + 128],
                    rc[(h, kt2)][:, ch * SC: ch * SC + SC],
                    start=(g == 0), stop=(g == 7))
            nc.any.tensor_copy(
                ret_out[:, mt * S + ch * SC: mt * S + ch * SC + SC], ps[:])


def build_nc():
    nc = bacc.Bacc(None, target_bir_lowering=False)

    inp = nc.dram_tensor("inp", [S, 3], dt.int32, kind="ExternalInput")
    qmat = nc.dram_tensor("qmat", [P_TAB, C], dt.float32, kind="ExternalInput")
    ce = nc.dram_tensor("ce", [C, D], dt.float32, kind="ExternalInput")
    de = nc.dram_tensor("de", [C, D], dt.float32, kind="ExternalInput")
    fe = nc.dram_tensor("fe", [C, D], dt.float32, kind="ExternalInput")
    mu = nc.dram_tensor("mu", [C, 1], dt.float32, kind="ExternalInput")
    re = nc.dram_tensor("re", [2, D], dt.float32, kind="ExternalInput")
    dwv = nc.dram_tensor("dwv", [2 * D, 1], dt.float32r, kind="ExternalInput")
    dbv = nc.dram_tensor("dbv", [1, 1], dt.float32, kind="ExternalInput")
    ident = nc.dram_tensor("ident", [128, 128], dt.float32, kind="ExternalInput")
    mask_i_x = nc.dram_tensor("mask_i", [128, 128], dt.float32, kind="ExternalInput")
    mask_s_x = nc.dram_tensor("mask_s", [128, 128], dt.float32, kind="ExternalInput")
    wx = {}
    for pre in ("qe", "ke", "kr"):
        for n in ("wq", "wk", "wv"):
            wx[f"{pre}_{n}"] = nc.dram_tensor(
                f"{pre}_{n}", [HL, D, D], dt.float32r, kind="ExternalInput")
        wx[f"{pre}_wo"] = nc.dram_tensor(
            f"{pre}_wo", [HL * D, D], dt.float32, kind="ExternalInput")
    out_ext = nc.dram_tensor("out", [1, S], dt.float32, kind="ExternalOutput")

    groups = [[0, 1], [2, 3], [4, 5], [6, 7]]

    from contextlib import ExitStack
    with tile.TileContext(nc) as tc, ExitStack() as es:
        const = es.enter_context(tc.tile_pool(name="const", bufs=1))
        wpool = es.enter_context(tc.tile_pool(name="wpool", bufs=1))
        stage = es.enter_context(tc.tile_pool(name="stage", bufs=2))
        act = es.enter_context(tc.tile_pool(name="act", bufs=1))
        work = es.enter_context(tc.tile_pool(name="work", bufs=2))
        ptp = es.enter_context(tc.tile_pool(name="ptp", bufs=4))
        psA = es.enter_context(tc.tile_pool(name="psA", bufs=2, space="PSUM"))
        psAcc = es.enter_context(tc.tile_pool(name="psAcc", bufs=4, space="PSUM"))
        psRow = es.enter_context(tc.tile_pool(name="psRow", bufs=2, space="PSUM"))
        dram = es.enter_context(tc.tile_pool(name="dram", bufs=1, space="DRAM"))
        pools = {"work": work, "pt": ptp, "psA": psA, "psAcc": psAcc,
                 "psRow": psRow}

        # ---------- constants ----------
        ident_sb = const.tile([128, 128], dt.float32)
        nc.sync.dma_start(ident_sb[:], ident[:])
        ones_col = const.tile([128, 1], dt.bfloat16)
        nc.vector.memset(ones_col[:], 1.0)
        ones_row = const.tile([1, SC], dt.bfloat16)
        nc.vector.memset(ones_row[:], 1.0)
        mask_i_sb = const.tile([128, 128], dt.bfloat16)
        mask_s_sb = const.tile([128, 128], dt.bfloat16)
        for m_sb, m_x in ((mask_i_sb, mask_i_x), (mask_s_sb, mask_s_x)):
            mst = stage.tile([128, 128], dt.float32, tag="mstage", bufs=1)
            nc.sync.dma_start(mst[:], m_x[:])
            nc.vector.tensor_copy(m_sb[:], mst[:])
        consts = {"ones_col": ones_col, "ones_row": ones_row,
                  "mask_i": mask_i_sb, "mask_s": mask_s_sb}

        # ---------- weights -> SBUF bf16 (per-MHA, shared tags) ----------
        def load_w(pre):
            w = {}
            for n in ("wq", "wk", "wv"):
                wb = wpool.tile([128, HL * 2 * 256], dt.float32r,
                                tag=n, name=f"wb_{pre}_{n}", bufs=2)
                nc.sync.dma_start(
                    wb[:].rearrange("p (h kt e) -> p h kt e", h=HL, kt=2),
                    wx[f"{pre}_{n}"][:].rearrange(
                        "h (kt p) e -> p h kt e", p=128))
                w[n] = wb
            st = stage.tile([128, 8 * 256], dt.float32, tag="wstage",
                            name=f"wst_{pre}_wo", bufs=1)
            nc.sync.dma_start(
                st[:, :8 * 256].rearrange("p (g d) -> p g d", g=8),
                wx[f"{pre}_wo"][:].rearrange("(g p) d -> p g d", p=128))
            wb = wpool.tile([128, 8 * 256], dt.bfloat16, tag="wo",
                            name=f"wb_{pre}_wo", bufs=2)
            nc.any.tensor_copy(wb[:], st[:, :8 * 256])
            w["wo"] = wb
            return w

        # ---------- embeds: ce2 = c + mu*d, fe2 = mu*f ----------
        de_sb = stage.tile([128, 2 * 256], dt.float32, tag="emb_d", bufs=1)
        ce_sb = stage.tile([128, 2 * 256], dt.float32, tag="emb_c", bufs=1)
        fe_sb = stage.tile([128, 2 * 256], dt.float32, tag="emb_f", bufs=1)
        mu_sb = const.tile([128, 2], dt.float32)
        for t_sb, t_x in ((de_sb, de), (ce_sb, ce), (fe_sb, fe)):
            nc.sync.dma_start(
                t_sb[:].rearrange("p (kt d) -> p kt d", kt=2),
                t_x[:].rearrange("(kt p) d -> p kt d", p=128))
        nc.sync.dma_start(
            mu_sb[:].rearrange("p (kt o) -> p kt o", kt=2),
            mu[:].rearrange("(kt p) o -> p kt o", p=128))
        ce2 = const.tile([128, 2 * 256], dt.bfloat16)
        fe2 = const.tile([128, 2 * 256], dt.bfloat16)
        tmp_md = stage.tile([128, 2 * 256], dt.float32, tag="emb_t", bufs=1)
        for kt in range(2):
            sl = slice(kt * 256, kt * 256 + 256)
            nc.vector.tensor_scalar_mul(tmp_md[:, sl], de_sb[:, sl],
                                        mu_sb[:, kt:kt + 1])
            nc.vector.tensor_tensor(out=ce2[:, sl], in0=tmp_md[:, sl],
                                    in1=ce_sb[:, sl], op=ALU.add)
            nc.vector.tensor_scalar_mul(tmp_md[:, sl], fe_sb[:, sl],
                                        mu_sb[:, kt:kt + 1])
            nc.vector.tensor_copy(fe2[:, sl], tmp_md[:, sl])

        # r_embed rows, d_W, d_b
        r0f = stage.tile([1, D], dt.float32, tag="r0f", bufs=1)
        r1f = stage.tile([1, D], dt.float32, tag="r1f", bufs=1)
        nc.sync.dma_start(r0f[:], re[0:1, :])
        nc.sync.dma_start(r1f[:], re[1:2, :])
        r0b = const.tile([1, D], dt.bfloat16)
        drb = const.tile([1, D], dt.bfloat16)
        nc.vector.tensor_copy(r0b[:], r0f[:])
        nc.vector.tensor_tensor(out=drb[:], in0=r1f[:], in1=r0f[:],
                                op=ALU.subtract)
        dw_b = const.tile([128, 4], dt.float32r)
        nc.sync.dma_start(
            dw_b[:].rearrange("p (i o) -> p i o", i=4),
            dwv[:].rearrange("(i p) o -> p i o", p=128))
        db_sb = const.tile([1, 1], dt.float32)
        nc.sync.dma_start(db_sb[:], dbv[:])

        # ---------- gather + transpose concept ----------
        concept_T = act.tile([128, 2 * S], dt.bfloat16, tag="conceptT")
        for t in range(NT):
            idx = stage.tile([128, 1], dt.int32, tag="idx")
            nc.sync.dma_start(idx[:], inp[t * 128:(t + 1) * 128, 0:1])
            idx0 = stage.tile([128, 1], dt.int32, tag="idx0")
            nc.vector.tensor_scalar_add(idx0[:], idx[:], -1)
            cn = stage.tile([128, C], dt.float32, tag="cnat")
            nc.gpsimd.indirect_dma_start(
                out=cn[:], out_offset=None, in_=qmat[:],
                in_offset=bass.IndirectOffsetOnAxis(ap=idx0[:, :1], axis=0))
            for kt in range(2):
                pt_ps = psA.tile([128, SC], dt.float32, tag="mm")
                nc.tensor.transpose(pt_ps[:, :128],
                                    cn[:, kt * 128:(kt + 1) * 128],
                                    ident_sb[:])
                nc.any.tensor_copy(
                    concept_T[:, kt * S + t * 128: kt * S + t * 128 + 128],
                    pt_ps[:, :128])

        # cnum row + corr row
        corr_i = stage.tile([1, S], dt.int32, tag="corri", bufs=1)
        nc.sync.dma_start(corr_i[:], inp[:].rearrange("s c -> c s")[2:3, :])
        corr_f = stage.tile([1, S], dt.float32, tag="corrf", bufs=1)
        nc.vector.tensor_copy(corr_f[:], corr_i[:])
        s1b = act.tile([1, S], dt.bfloat16, tag="s1b")
        s2b = act.tile([1, S], dt.bfloat16, tag="s2b")
        for ch in range(NCH):
            psr = psA.tile([1, SC], dt.float32, tag="mm", name="psr_cnum")
            for kt in range(2):
                nc.tensor.matmul(
                    psr[:], ones_col[:],
                    concept_T[:, kt * S + ch * SC: kt * S + ch * SC + SC],
                    start=(kt == 0), stop=(kt == 1))
            nc.vector.tensor_copy(s1b[:, ch * SC: ch * SC + SC], psr[:])
            nc.vector.tensor_tensor(
                out=s2b[:, ch * SC: ch * SC + SC],
                in0=corr_f[:, ch * SC: ch * SC + SC], in1=psr[:],
                op=ALU.mult)

        # ---------- x_T, y_T ----------
        x_T = act.tile([128, 2 * S], dt.float32r, tag="xT")
        y_T = act.tile([128, 2 * S], dt.float32r, tag="yT")
        for mt in range(2):
            for ch in range(NCH):
                ps = psA.tile([128, SC], dt.float32, tag="mm")
                for kt in range(2):
                    nc.tensor.matmul(
                        ps[:],
                        ce2[:, kt * 256 + mt * 128: kt * 256 + mt * 128 + 128],
                        concept_T[:, kt * S + ch * SC: kt * S + ch * SC + SC],
                        start=(kt == 0), stop=(kt == 1))
                nc.any.tensor_copy(
                    x_T[:, mt * S + ch * SC: mt * S + ch * SC + SC], ps[:])
                ps2 = psA.tile([128, SC], dt.float32, tag="mm")
                for kt in range(2):
                    nc.tensor.matmul(
                        ps2[:],
                        fe2[:, kt * 256 + mt * 128: kt * 256 + mt * 128 + 128],
                        concept_T[:, kt * S + ch * SC: kt * S + ch * SC + SC],
                        start=(kt == 0), stop=False)
                nc.tensor.matmul(
                    ps2[:], r0b[0:1, mt * 128: mt * 128 + 128],
                    s1b[0:1, ch * SC: ch * SC + SC], start=False, stop=False)
                nc.tensor.matmul(
                    ps2[:], drb[0:1, mt * 128: mt * 128 + 128],
                    s2b[0:1, ch * SC: ch * SC + SC], start=False, stop=True)
                nc.any.tensor_copy(
                    y_T[:, mt * S + ch * SC: mt * S + ch * SC + SC], ps2[:])

        # ---------- qe / ke encoders + pair AllReduce ----------
        xhat_T = act.tile([128, 2 * S], dt.float32r, tag="xhatT")
        yhat_T = act.tile([128, 2 * S], dt.float32r, tag="yhatT")
        for (src, wpre, hat) in ((x_T, "qe", xhat_T), (y_T, "ke", yhat_T)):
            w = load_w(wpre)
            part = stage.tile([128, 2 * S], dt.float32r, tag="part", bufs=1)
            _mha(nc, tc, pools, consts, src, src,
                 w["wq"], w["wk"], w["wv"], w["wo"],
                 strict=False, ret_out=part)
            b_in = dram.tile([128, 2 * S], dt.float32r, tag=f"bin_{wpre}")
            b_out = dram.tile([128, 2 * S], dt.float32r, tag=f"bout_{wpre}")
            nc.sync.dma_start(b_in[:], part[:])
            nc.gpsimd.collective_compute(
                "AllReduce", ALU.add, replica_groups=groups,
                ins=[b_in[:].opt()], outs=[b_out[:].opt()])
            nc.sync.dma_start(hat[:], b_out[:])

        # ---------- kr ----------
        w = load_w("kr")
        okr = stage.tile([128, 2 * S], dt.float32r, tag="part", bufs=1)
        _mha(nc, tc, pools, consts, xhat_T, yhat_T,
             w["wq"], w["wk"], w["wv"], w["wo"],
             strict=True, ret_out=okr)

        # ---------- final head: t1 (AllReduced) + t2, sigmoid ----------
        t1f = stage.tile([1, S], dt.float32, tag="t1f", bufs=1)
        for ch in range(NCH):
            psr = psA.tile([1, SC], dt.float32, tag="mm", name="psr_t1")
            for kt in range(2):
                nc.tensor.matmul(
                    psr[:], dw_b[:, kt:kt + 1],
                    okr[:, kt * S + ch * SC: kt * S + ch * SC + SC],
                    start=(kt == 0), stop=(kt == 1))
            nc.vector.tensor_copy(t1f[:, ch * SC: ch * SC + SC], psr[:])
        tb_in = dram.tile([1, S], dt.float32, tag="bin_t1")
        tb_out = dram.tile([1, S], dt.float32, tag="bout_t1")
        nc.sync.dma_start(tb_in[:], t1f[:])
        nc.gpsimd.collective_compute(
            "AllReduce", ALU.add, replica_groups=groups,
            ins=[tb_in[:].opt()], outs=[tb_out[:].opt()])
        t1full = stage.tile([1, S], dt.float32, tag="t1full", bufs=1)
        nc.sync.dma_start(t1full[:], tb_out[:])

        pred = stage.tile([1, S], dt.float32, tag="pred", bufs=1)
        for ch in range(NCH):
            psr = psA.tile([1, SC], dt.float32, tag="mm", name="psr_t2")
            for kt in range(2):
                nc.tensor.matmul(
                    psr[:], dw_b[:, 2 + kt: 3 + kt],
                    xhat_T[:, kt * S + ch * SC: kt * S + ch * SC + SC],
                    start=(kt == 0), stop=(kt == 1))
            ssum = stage.tile([1, SC], dt.float32, tag="ssum", bufs=1)
            nc.vector.tensor_tensor(
                out=ssum[:], in0=t1full[:, ch * SC: ch * SC + SC], in1=psr[:],
                op=ALU.add)
            nc.scalar.activation(pred[:, ch * SC: ch * SC + SC], ssum[:],
                                 AF.Sigmoid, bias=db_sb[0:1, 0:1])
        nc.sync.dma_start(out_ext[:], pred[:])

    nc.finalize()
    return nc


_NC_CACHE = None


def _get_nc():
    global _NC_CACHE
    if _NC_CACHE is None:
        _NC_CACHE = build_nc()
    return _NC_CACHE


def make_in_maps(inputs):
    f32 = np.float32
    common = {
        "qmat": np.ascontiguousarray(np.asarray(inputs["Q_matrix"], f32)),
        "ce": np.asarray(inputs["c_embed"], f32),
        "de": np.asarray(inputs["d_embed"], f32),
        "fe": np.asarray(inputs["f_embed"], f32),
        "mu": np.asarray(inputs["mu_q"], f32),
        "re": np.asarray(inputs["r_embed"], f32),
        "dwv": np.asarray(inputs["d_W"], f32),
        "dbv": np.asarray(inputs["d_b"], f32).reshape(1, 1),
        "ident": np.eye(128, dtype=f32),
        "mask_i": np.triu(np.ones((128, 128), f32), 0),
        "mask_s": np.triu(np.ones((128, 128), f32), 1),
    }
    inp_all = np.asarray(inputs["inputs"], np.int32)
    in_maps = []
    for c in range(8):
        b, h0 = c // 2, (c % 2) * HL
        m = dict(common)
        m["inp"] = np.ascontiguousarray(inp_all[b])
        for pre in ("qe", "ke", "kr"):
            m[f"{pre}_wq"] = np.ascontiguousarray(
                np.asarray(inputs[f"{pre}_wQ"], f32)[h0:h0 + HL])
            m[f"{pre}_wk"] = np.ascontiguousarray(
                np.asarray(inputs[f"{pre}_wK"], f32)[h0:h0 + HL])
            m[f"{pre}_wv"] = np.ascontiguousarray(
                np.asarray(inputs[f"{pre}_wV"], f32)[h0:h0 + HL])
            m[f"{pre}_wo"] = np.ascontiguousarray(
                np.asarray(inputs[f"{pre}_wO"], f32)[h0 * D:(h0 + HL) * D])
        in_maps.append(m)
    return in_maps


def kernel(**inputs):
    nc = _get_nc()
    in_maps = make_in_maps(inputs)
    res = run_bass_kernel_spmd(nc, in_maps, core_ids=list(range(8)))
    outs = res.results
    pred = np.stack([outs[2 * b]["out"].reshape(S) for b in range(B)])
    return pred[..., None].astype(np.float32)
